# revision 17
# baseline (speedup 1.0000x reference)
"""Trainium2 Bass kernel for the BKT (multi-HMM knowledge tracing) forward model.

Reformulated recursion (validated in proto.py):
  state  γ(t) = α(t) − a3(t−1)            [128 students × (s,k) = 200]
  chain  x4(t) = u4(t) + κ_{t−1}·a3(t−1)  [stt w4, DVE]
         ps2(t)[s'] = Σ_s exp(x4[s,s'])   [2× ACT Exp w2 with accum_out]
         a3(t) = ln(ps2)                  [ACT Ln w2]
  off    pd(t) = a3(t−1) − a3(t)          [Pool tt w2]
         γ(t+1) = cinv_t ⊙ (γ(t)+pd(t))   [2× DVE stt w100]
         r'(t+2) = <Q_{t+1}, γ(t+1)> + σ_{t+1}·a3(t)
             [2× DVE affine_mul_reduce: accum = Σ(γ+a3)·Q = <Q,γ> + a3·ΣQ
              and ΣQ_{t+1} = σ_{t+1}, so the seed rides the bias slot]
         u4(t+2) = w4(t+2) + r'(t+2)      [Pool tt w4]
  where Q_t = c_{t+1}⊙(1−c_t), σ_t = ΣQ_t, κ_t = 1−σ_t, all host streams.
  (tensor_tensor_reduce and ACT accum_out crash this runtime — avoided.)
Outputs deferred to one batched tail:
  D(j) = (r'_1−r'_0)(j) + κ_{j−1}·(a3_1−a3_0)(j−1),
  out[t,o] = L0[t,o] + ln(1+e^{dL[t,o]+D}) − ln(1+e^{D}).
Step 0/1 warm-started on host (uploads γ(1), a3(0), u4(1..2), r'(2), D(0..1)).
No gathers: Q/cinv streamed dense from HBM, chunked + double buffered.
"""

import os
from contextlib import ExitStack

import numpy as np

N_PROBLEMS = 10000
N_KCS = 100
BATCH = 1024
T_FULL = 500
N_CORES = 8
BL = BATCH // N_CORES  # 128 students per core

_CH = 32       # time steps per Q/cinv stream chunk
_PREF = 2      # chunks prefetched ahead
_SMALL_ON_POOL = True


def _log_softmax(x, axis):
    x = x.astype(np.float64)
    m = x.max(axis=axis, keepdims=True)
    e = np.exp(x - m)
    return (x - m) - np.log(e.sum(axis=axis, keepdims=True))


def _setup_act_tables():
    """Force the 'natural_log_exp_and_others' ACT table set to be chosen for
    both Exp and Ln so no ACT_TABLE_LOAD appears mid-loop."""
    import glob
    import json
    import tempfile

    if os.environ.get("_BKT_ACT_TABLES"):
        return
    from neuronxcc.driver.Job import Job  # pyright: ignore[reportMissingImports]
    from neuronxcc.driver.jobs.support.FindActInfo import (  # pyright: ignore[reportMissingImports]
        findActInfoFile,
    )

    src = findActInfoFile(Job.getPackageDir(), "gen3")
    d = json.load(open(src))
    d["act_func_sets"] = sorted(
        d["act_func_sets"],
        key=lambda s: s["name"] != "natural_log_exp_and_others")
    tmp = tempfile.mkdtemp(prefix="bkt_act_")
    with open(tmp + "/act_info.json", "w") as f:
        json.dump(d, f)
    for p in glob.glob(os.path.dirname(src) + "/*"):
        b = os.path.basename(p)
        if b != "act_info.json":
            os.symlink(p, tmp + "/" + b)
    os.environ["BASS_ACT_ROOT_JSON_PATH"] = tmp + "/act_info.json"
    os.environ["_BKT_ACT_TABLES"] = "1"

    import concourse.bacc as bacc_mod
    import concourse.mybir as mybir

    def tables(arch):
        return {
            e["name"]: {mybir.ActivationFunctionType.from_pwp(v)
                        for v in e["act"].keys()}
            for e in d["act_func_sets"]
        }

    bacc_mod.get_activation_tables = tables


def _emit_program(T):
    import concourse.mybir as mybir
    import concourse.tile as tile
    from concourse import bacc, library_config

    _setup_act_tables()

    f32 = mybir.dt.float32
    Alu = mybir.AluOpType
    Act = mybir.ActivationFunctionType
    K = N_KCS

    nc = bacc.Bacc("TRN2", target_bir_lowering=False, debug=False)

    # DRAM inputs
    f16 = mybir.dt.float16
    qs = nc.dram_tensor("qs", [BL, T * K], f16, kind="ExternalInput")
    cinvs = nc.dram_tensor("cinvs", [BL, T * K], f16, kind="ExternalInput")
    sigs = nc.dram_tensor("sigs", [BL, T], f32, kind="ExternalInput")
    sigd = nc.dram_tensor("sigd", [BL, T], f32, kind="ExternalInput")
    w4s = nc.dram_tensor("w4s", [BL, 4 * T], f32, kind="ExternalInput")
    kaps = nc.dram_tensor("kaps", [BL, T], f32, kind="ExternalInput")
    l0s_d = nc.dram_tensor("l0s", [BL, 2 * T], f32, kind="ExternalInput")
    dls_d = nc.dram_tensor("dls", [BL, 2 * T], f32, kind="ExternalInput")
    gammaw = nc.dram_tensor("gammaw", [BL, 2 * K], f16, kind="ExternalInput")
    a30w = nc.dram_tensor("a30w", [BL, 2], f32, kind="ExternalInput")
    u4w = nc.dram_tensor("u4w", [BL, 8], f32, kind="ExternalInput")
    rp2w = nc.dram_tensor("rp2w", [BL, 2], f32, kind="ExternalInput")
    d01w = nc.dram_tensor("d01w", [BL, 2], f32, kind="ExternalInput")
    out_d = nc.dram_tensor("out", [BL, 2 * T], f32, kind="ExternalOutput")

    n_chunks = (T + _CH - 1) // _CH

    with ExitStack() as ctx:
        tc = ctx.enter_context(tile.TileContext(nc))
        if _SMALL_ON_POOL:
            nc.gpsimd.load_library(library_config.standard)

        fixed = ctx.enter_context(tc.tile_pool(name="fixed", bufs=1))
        slabp = ctx.enter_context(tc.tile_pool(name="slabs", bufs=_PREF + 1))
        gpool = ctx.enter_context(tc.tile_pool(name="gamma", bufs=2))
        x4p = ctx.enter_context(tc.tile_pool(name="x4", bufs=3))
        sm = ctx.enter_context(tc.tile_pool(name="sm", bufs=3))

        # --- fixed tiles + warm uploads (issue order = DMA drain order) ---
        gamma = gpool.tile([BL, 2 * K], f16, tag="gamma")
        nc.sync.dma_start(gamma[:], gammaw.ap())
        a3buf = fixed.tile([BL, 2 * T], f32)
        nc.sync.dma_start(a3buf[:, 0:2], a30w.ap())
        u4buf = fixed.tile([BL, 4 * T], f32)
        nc.sync.dma_start(u4buf[:, 4:12], u4w.ap())
        r2buf = fixed.tile([BL, 2 * T], f32)
        nc.sync.dma_start(r2buf[:, 4:6], rp2w.ap())
        kap = fixed.tile([BL, T], f32)
        nc.sync.dma_start(kap[:], kaps.ap())
        sig = fixed.tile([BL, T], f32)
        nc.sync.dma_start(sig[:], sigs.ap())
        sigdt = fixed.tile([BL, T], f32)
        nc.sync.dma_start(sigdt[:], sigd.ap())

        qslab = [None] * n_chunks
        cislab = [None] * n_chunks

        def issue_chunk(n):
            t0 = n * _CH
            w = min(_CH, T - t0) * K
            qt = slabp.tile([BL, _CH, K], f16, tag="qsl")
            nc.sync.dma_start(qt[:].rearrange("p a b -> p (a b)")[:, 0:w],
                              qs.ap()[:, t0 * K:t0 * K + w])
            ct = slabp.tile([BL, _CH, K], f16, tag="cisl")
            nc.sync.dma_start(ct[:].rearrange("p a b -> p (a b)")[:, 0:w],
                              cinvs.ap()[:, t0 * K:t0 * K + w])
            qslab[n], cislab[n] = qt, ct

        for n in range(min(_PREF, n_chunks)):
            issue_chunk(n)

        w4b = fixed.tile([BL, 4 * T], f32)
        nc.sync.dma_start(w4b[:], w4s.ap())
        Dbuf = fixed.tile([BL, T], f32)
        nc.sync.dma_start(Dbuf[:, 0:2], d01w.ap())
        l0b = fixed.tile([BL, 2 * T], f32)
        nc.sync.dma_start(l0b[:], l0s_d.ap())
        dlb = fixed.tile([BL, 2 * T], f32)
        nc.sync.dma_start(dlb[:], dls_d.ap())

        junk = fixed.tile([BL, 2 * K], f16)

        smalls = nc.gpsimd if _SMALL_ON_POOL else nc.vector

        # --- main loop: chain t = 1..T-2; off-chain pipelined one iter late ---
        gamma_box = [gamma]

        def emit_off(t):
            n = t // _CH
            pd2t = sm.tile([BL, 2], f32, tag="pd2")
            smalls.tensor_tensor(
                out=pd2t[:], in0=a3buf[:, 2 * (t - 1):2 * t],
                in1=a3buf[:, 2 * t:2 * t + 2], op=Alu.subtract)
            gold = gamma_box[0]
            gnew = gpool.tile([BL, 2 * K], f16, tag="gamma")
            ci = cislab[n][:, t % _CH, :]
            qv = qslab[n][:, t % _CH, :]
            for s in range(2):
                nc.vector.scalar_tensor_tensor(
                    out=gnew[:, s * K:(s + 1) * K],
                    in0=gold[:, s * K:(s + 1) * K],
                    scalar=pd2t[:, s:s + 1],
                    in1=ci, op0=Alu.add, op1=Alu.mult,
                )
            h2t = sm.tile([BL, 2], f32, tag="h2")
            smalls.tensor_tensor(
                out=h2t[:], in0=a3buf[:, 2 * t:2 * t + 2],
                in1=sig[:, t:t + 1].broadcast_to([BL, 2]), op=Alu.mult)
            for s in range(2):
                # raw dot: accum_out = <Q_{t+1}, γ_s(t+1)>
                nc.vector.scalar_tensor_tensor(
                    out=junk[:, s * K:(s + 1) * K],
                    in0=gnew[:, s * K:(s + 1) * K], scalar=0.0, in1=qv,
                    op0=Alu.bypass, op1=Alu.mult,
                    accum_out=r2buf[:, 2 * (t + 2) + s:2 * (t + 2) + s + 1],
                )
            u4a = sm.tile([BL, 4], f32, tag="u4a")
            smalls.tensor_tensor(
                out=u4a[:].rearrange("p (s sp) -> p s sp", s=2),
                in0=w4b[:, 4 * (t + 2):4 * (t + 2) + 4]
                    .rearrange("p (s sp) -> p s sp", s=2),
                in1=r2buf[:, 2 * (t + 2):2 * (t + 2) + 2]
                    .rearrange("p (s o) -> p s o", s=2).broadcast_to([BL, 2, 2]),
                op=Alu.add,
            )
            smalls.tensor_tensor(
                out=u4buf[:, 4 * (t + 2):4 * (t + 2) + 4]
                    .rearrange("p (s sp) -> p s sp", s=2),
                in0=u4a[:].rearrange("p (s sp) -> p s sp", s=2),
                in1=h2t[:].rearrange("p (s o) -> p s o", s=2)
                    .broadcast_to([BL, 2, 2]),
                op=Alu.add,
            )
            gamma_box[0] = gnew

        pending = None
        for t in range(1, T - 1):
            n = t // _CH
            m = n + _PREF - 1
            if t % _CH == 0 and m < n_chunks and qslab[m] is None:
                issue_chunk(m)

            # CHAIN: x4(t) = κ_{t-1}·a3(t-1) + u4(t)
            x4t = x4p.tile([BL, 4], f32, tag="x4")
            nc.vector.scalar_tensor_tensor(
                out=x4t[:].rearrange("p (s sp) -> p s sp", s=2),
                in0=a3buf[:, 2 * (t - 1):2 * t]
                    .rearrange("p (s o) -> p s o", s=2).broadcast_to([BL, 2, 2]),
                scalar=kap[:, t:t + 1],
                in1=u4buf[:, 4 * t:4 * t + 4].rearrange("p (s sp) -> p s sp", s=2),
                op0=Alu.mult, op1=Alu.add,
            )
            if pending is not None:
                emit_off(pending)
                pending = None
            e4t = sm.tile([BL, 4], f32, tag="e4")
            nc.scalar.activation(e4t[:], x4t[:], Act.Exp)
            ps2t = sm.tile([BL, 2], f32, tag="ps2")
            nc.vector.tensor_tensor(
                out=ps2t[:], in0=e4t[:, 0:2], in1=e4t[:, 2:4], op=Alu.add)
            nc.scalar.activation(a3buf[:, 2 * t:2 * t + 2], ps2t[:], Act.Ln)
            if t <= T - 3:
                pending = t
        if pending is not None:
            emit_off(pending)

        # --- deferred output tail ---
        da = fixed.tile([BL, T], f32)
        a3v = a3buf[:].rearrange("p (t s) -> p t s", s=2)
        nc.vector.tensor_tensor(
            out=da[:, 0:T - 1].rearrange("p (t o) -> p t o", o=1),
            in0=a3v[:, 0:T - 1, 1:2], in1=a3v[:, 0:T - 1, 0:1],
            op=Alu.subtract)
        dr = fixed.tile([BL, T], f32)
        r2v = r2buf[:].rearrange("p (t s) -> p t s", s=2)
        nc.vector.tensor_tensor(
            out=dr[:, 2:T].rearrange("p (t o) -> p t o", o=1),
            in0=r2v[:, 2:T, 1:2], in1=r2v[:, 2:T, 0:1], op=Alu.subtract)
        m1 = fixed.tile([BL, T], f32)
        nc.vector.tensor_tensor(
            out=m1[:, 2:T], in0=kap[:, 2:T], in1=da[:, 1:T - 1], op=Alu.mult)
        m2 = fixed.tile([BL, T], f32)
        nc.vector.tensor_tensor(
            out=m2[:, 2:T], in0=sigdt[:, 2:T], in1=da[:, 0:T - 2], op=Alu.mult)
        d1t = fixed.tile([BL, T], f32)
        nc.vector.tensor_tensor(
            out=d1t[:, 2:T], in0=dr[:, 2:T], in1=m1[:, 2:T], op=Alu.add)
        nc.vector.tensor_tensor(
            out=Dbuf[:, 2:T], in0=d1t[:, 2:T], in1=m2[:, 2:T], op=Alu.add)
        yD = fixed.tile([BL, 2 * T], f32)
        nc.vector.tensor_tensor(
            out=yD[:].rearrange("p (t o) -> p t o", o=2),
            in0=dlb[:].rearrange("p (t o) -> p t o", o=2),
            in1=Dbuf[:].rearrange("p (t o) -> p t o", o=1)
                .broadcast_to([BL, T, 2]),
            op=Alu.add)
        e2 = fixed.tile([BL, 2 * T], f32)
        nc.scalar.activation(e2[:], yD[:], Act.Exp)
        l2 = fixed.tile([BL, 2 * T], f32)
        nc.scalar.activation(l2[:], e2[:], Act.Ln, bias=1.0)
        ed = fixed.tile([BL, T], f32)
        nc.scalar.activation(ed[:], Dbuf[:], Act.Exp)
        ld = fixed.tile([BL, T], f32)
        nc.scalar.activation(ld[:], ed[:], Act.Ln, bias=1.0)
        outb = fixed.tile([BL, 2 * T], f32)
        nc.vector.tensor_tensor(
            out=outb[:].rearrange("p (t o) -> p t o", o=2),
            in0=l2[:].rearrange("p (t o) -> p t o", o=2),
            in1=ld[:].rearrange("p (t o) -> p t o", o=1)
                .broadcast_to([BL, T, 2]),
            op=Alu.subtract)
        nc.vector.tensor_tensor(out=outb[:], in0=outb[:], in1=l0b[:], op=Alu.add)
        nc.sync.dma_start(out_d.ap(), outb[:])

    nc.compile()
    return nc


def _prep_inputs(corr, kc, problem, A, trans_logits, obs_logits_problem,
                 init_logits, T):
    corr = np.asarray(corr).astype(np.int64)[:, :T]
    kc = np.asarray(kc).astype(np.int64)[:, :T]
    problem = np.asarray(problem).astype(np.int64)[:, :T]
    A = np.asarray(A).astype(np.float64)
    K = N_KCS

    log_t = _log_softmax(np.asarray(trans_logits), axis=1)
    G = A @ log_t.reshape(K, 4)                       # [P,4], j = 2 s' + s
    L = _log_softmax(np.asarray(obs_logits_problem), axis=2)
    la0 = _log_softmax(np.asarray(init_logits), axis=1)

    in_maps = []
    for i in range(N_CORES):
        sl = slice(i * BL, (i + 1) * BL)
        kc_l, pp_l, cr_l = kc[sl], problem[sl], corr[sl]
        c = A[kc_l]                                   # [BL,T,K]
        cinv = 1.0 - c
        Q = c[:, 1:] * cinv[:, :-1]                   # Q[:,j] = c_{j+1}*cinv_j
        sigma = Q.sum(-1)
        kappa = 1.0 - sigma
        OLL = np.take_along_axis(
            L[pp_l], cr_l[:, :, None, None], axis=3)[:, :, :, 0]  # [BL,T,2]
        Gk = G[kc_l]
        w4 = np.stack([Gk[..., 0] + OLL[..., 0], Gk[..., 2] + OLL[..., 0],
                       Gk[..., 1] + OLL[..., 1], Gk[..., 3] + OLL[..., 1]],
                      axis=2)                         # [BL,T,4] (s,s')
        Lp = L[pp_l]
        l0 = np.ascontiguousarray(Lp[:, :, 0, :]).reshape(BL, 2 * T)
        dl = np.ascontiguousarray(Lp[:, :, 1, :] - Lp[:, :, 0, :]).reshape(BL, 2 * T)

        # warm start
        alpha0 = np.broadcast_to(la0.T.reshape(1, 2, K), (BL, 2, K))
        a2_0 = np.einsum('bk,bsk->bs', c[:, 0], alpha0)
        x40 = w4[:, 0].reshape(BL, 2, 2) + a2_0[:, :, None]
        a30 = np.log(np.exp(x40).sum(axis=1))
        alpha1 = cinv[:, 0][:, None, :] * alpha0 + c[:, 0][:, None, :] * a30[:, :, None]
        gamma1 = alpha1 - a30[:, :, None]
        a2_1 = np.einsum('bk,bsk->bs', c[:, 1], gamma1) + a30
        u4_1 = w4[:, 1].reshape(BL, 2, 2) + a2_1[:, :, None] \
            - kappa[:, 0][:, None, None] * a30[:, :, None]
        rp2_raw = np.einsum('bk,bsk->bs', Q[:, 1], gamma1)
        rp2 = rp2_raw + sigma[:, 1][:, None] * a30
        u4_2 = w4[:, 2].reshape(BL, 2, 2) + rp2[:, :, None]

        # iteration-shifted streams
        kap_it = np.zeros((BL, T)); kap_it[:, 1:] = kappa[:, 0:T - 1]
        sig_it = np.zeros((BL, T)); sig_it[:, 1:T - 2] = sigma[:, 2:T - 1]
        sig_d = np.zeros((BL, T)); sig_d[:, 2:] = sigma[:, 1:T - 1]
        q_it = np.zeros((BL, T, K)); q_it[:, 1:T - 2] = Q[:, 2:T - 1]
        cinv_it = np.zeros((BL, T, K)); cinv_it[:, 1:T - 2] = cinv[:, 1:T - 2]

        f = np.float32
        im = {
            "qs": q_it.reshape(BL, T * K).astype(np.float16),
            "cinvs": cinv_it.reshape(BL, T * K).astype(np.float16),
            "w4s": w4.reshape(BL, 4 * T).astype(f),
            "kaps": kap_it.astype(f),
            "sigs": sig_it.astype(f),
            "sigd": sig_d.astype(f),
            "l0s": l0.astype(f),
            "dls": dl.astype(f),
            "gammaw": gamma1.reshape(BL, 2 * K).astype(np.float16),
            "a30w": a30.astype(f),
            "u4w": np.concatenate([u4_1.reshape(BL, 4), u4_2.reshape(BL, 4)],
                                  axis=1).astype(f),
            "rp2w": rp2_raw.astype(f),
            "d01w": np.stack([a2_0[:, 1] - a2_0[:, 0],
                              a2_1[:, 1] - a2_1[:, 0]], axis=1).astype(f),
        }
        in_maps.append(im)
    return in_maps


def kernel(corr, kc, problem, A, trans_logits, obs_logits_problem, init_logits,
           _T=None, _trace=False):
    T = _T or T_FULL
    nc = _emit_program(T)
    in_maps = _prep_inputs(corr, kc, problem, A, trans_logits,
                           obs_logits_problem, init_logits, T)

    from concourse.bass_utils import run_bass_kernel_spmd
    res = run_bass_kernel_spmd(nc, in_maps, core_ids=list(range(N_CORES)),
                               trace=_trace)
    outs = [r["out"].reshape(BL, T, 2) for r in res.results]
    full = np.concatenate(outs, axis=0).astype(np.float32)
    kernel.last_results = res
    return full


if __name__ == "__main__":
    pass
